# revision 4
# baseline (speedup 1.0000x reference)
"""Sequence-parallel self-attention for 8 TRN2 cores — minimal wire bytes.

Reference (N=8192, D=256, fp32):
    q = x @ WQ; k = x @ WK; v = x @ WV
    out = softmax(q @ k.T) @ v

The graded metric is wall-clock of kernel(), dominated by host->device
transfer over the axon tunnel (which compresses, so random fp32 is ~4x
the wire cost of fp16) plus per-call jit/compile overhead. Design:

  * ONE packed fp16 input per core (0.53 MiB vs 17.8 MiB baseline):
      xin [272, 1024] = [ x.T columns for my 1024 rows  (256 rows)
                        ; my 64-row shard of W = concat(WQ@WK.T, WV),
                          reshaped [64,256]->[16,1024]                ]
    Full x is assembled on-device with a single 8-core AllGather.
  * Output outT [256, 1024] fp16 (out.T for my rows).
  * JAX persistent compilation cache: run_bass_kernel_spmd re-jits a
    fresh closure every call, which otherwise re-runs the whole
    BIR->NEFF backend (~0.25s/call).

Per-core algebra (scores stay transposed so the softmax k-reduction is a
partition-axis ones-matmul):
    M  = (WQ @ WK.T).T @ xT_local                    [256, 1024]
    per k-chunk c (64 chunks of 128):
      scoresT = x_c @ M                              [128, 1024]
      expT    = exp(scoresT - 15)   (bf16)
      sums   += ones.T @ expT                        [1, 1024]
      UT     += x_c.T @ expT                         [256, 1024]
    outT = WV.T @ (UT / sums)                        [256, 1024]

x_c.T (natural-layout chunks, the UT lhsT) is derived on-chip from the
gathered x.T via X-bar DMA transpose (fp16, DRAM->SBUF) + a DVE cast to
bf16 (to match expT; expT can't be fp16 — exp(s-15) reaches ~1.2e11).

Numerics: fp16 score path end-to-end measures 3.3e-3 rel err in a
bit-accurate numpy model (bf16 would be 2.3e-2, over the 2e-2 gate).
All PSUM accumulation fp32.
"""

import numpy as np

N, D, P = 8192, 256, 8
NL = N // P          # 1024 q-rows per core
KC = 128             # k-chunk size
SB = 8               # k-chunks per superblock (= one AllGather block)
NSB = N // (KC * SB)  # 8 superblocks
WS = 2 * D // P      # 64 W-rows per core
XR = D + WS * D // 1024  # 272 packed input rows
EXP_SHIFT = -15.0

_CACHE = {}


def _config_jax_cache():
    if _CACHE.get("jax_cfg"):
        return
    _CACHE["jax_cfg"] = True
    try:
        import jax
        if jax.config.jax_compilation_cache_dir is None:
            jax.config.update("jax_compilation_cache_dir",
                              "/tmp/.bass_jax_comp_cache")
            jax.config.update("jax_persistent_cache_min_compile_time_secs", 0.0)
            jax.config.update("jax_persistent_cache_min_entry_size_bytes", 0)
    except Exception:
        pass


def _build():
    import concourse.bacc as bacc
    import concourse.mybir as mybir
    import concourse.tile as tile

    f32 = mybir.dt.float32
    f32r = mybir.dt.float32r
    f16 = mybir.dt.float16
    bf16 = mybir.dt.bfloat16
    EXP = mybir.ActivationFunctionType.Exp

    nc = bacc.Bacc("TRN2", target_bir_lowering=False, debug=False,
                   enable_asserts=False, num_devices=P)

    xin = nc.dram_tensor("xin", [XR, NL], f16, kind="ExternalInput").ap()
    outT = nc.dram_tensor("outT", [D, NL], f16, kind="ExternalOutput").ap()

    rg = [list(range(P))]
    BYP = mybir.AluOpType.bypass

    with tile.TileContext(nc) as tc:
        with (
            tc.tile_pool(name="dram", bufs=1, space="DRAM") as dpool,
            tc.tile_pool(name="const", bufs=1) as cpool,
            tc.tile_pool(name="proj", bufs=1) as ppool,
            tc.tile_pool(name="xts", bufs=4) as xtpool,
            tc.tile_pool(name="xfs", bufs=2) as xfpool,
            tc.tile_pool(name="xns", bufs=4) as xnpool,
            tc.tile_pool(name="expt", bufs=8) as epool,
            tc.tile_pool(name="tail", bufs=1) as tpool,
            tc.tile_pool(name="ps_scores", bufs=2, space="PSUM") as ps_s,
            tc.tile_pool(name="ps_ut", bufs=1, space="PSUM") as ps_ut,
            tc.tile_pool(name="ps_sums", bufs=1, space="PSUM") as ps_sum,
        ):
            # ---- AllGather the packed shard into full-x DRAM ----
            x_b = dpool.tile([XR, NL], f16, name="x_b")
            x_g = dpool.tile([P * XR, NL], f16, addr_space="Shared", name="x_g")
            nc.sync.dma_start(x_b[:], xin[:])
            nc.gpsimd.collective_compute(
                "AllGather", BYP, replica_groups=rg,
                ins=[x_b.opt()], outs=[x_g.opt()])

            # ---- weights from the gathered blocks ----
            # W row r (256 wide) lives in shard c=r//64's block, rows
            # [c*XR+256 : c*XR+272] as [16, 1024] (4 W-rows per row).
            wkqt_t = [cpool.tile([128, D], f16, tag=f"wkqt{h}", name=f"wkqt{h}")
                      for h in range(2)]
            wv_t = [cpool.tile([128, D], f16, tag=f"wv{h}", name=f"wv{h}")
                    for h in range(2)]
            for t, base in ((wkqt_t, 0), (wv_t, D)):
                for kp in range(2):
                    for s in range(2):
                        c = (base + kp * 128) // WS + s
                        nc.sync.dma_start(
                            t[kp][s * 64:(s + 1) * 64, :],
                            x_g[c * XR + D:(c + 1) * XR, :]
                            .rearrange("p (a d) -> (p a) d", a=4))

            xtl_t = [cpool.tile([128, NL], f16, tag=f"xtl{h}", name=f"xtl{h}")
                     for h in range(2)]
            for h in range(2):
                nc.sync.dma_start(xtl_t[h][:], xin[h * 128:(h + 1) * 128, :])
            ones_col = cpool.tile([128, 1], bf16, name="ones_col")
            ones_row = cpool.tile([1, 128], f32, name="ones_row")
            bias_t = cpool.tile([128, 1], f32, name="bias_t")
            nc.gpsimd.memset(ones_col[:], 1.0)
            nc.vector.memset(ones_row[:], 1.0)
            nc.vector.memset(bias_t[:], EXP_SHIFT)

            # ---- M = WKQ @ xT_local  (lhsT = WKQT = WQ @ WK.T) ----
            m_t = [ppool.tile([128, NL], f16, tag=f"m{h}", name=f"m{h}")
                   for h in range(2)]
            for mh in range(2):
                for nh in range(2):
                    pp = ps_s.tile([128, 512], f32, tag="scores", name="scores")
                    for kp in range(2):
                        nc.tensor.matmul(
                            pp[:],
                            wkqt_t[kp][:, mh * 128:(mh + 1) * 128],
                            xtl_t[kp][:, nh * 512:(nh + 1) * 512],
                            start=(kp == 0), stop=(kp == 1),
                        )
                    nc.vector.tensor_copy(
                        m_t[mh][:, nh * 512:(nh + 1) * 512], pp[:])

            # ---- persistent accumulators ----
            ut_ps = [ps_ut.tile([128, NL], f32, tag=f"ut{h}", name=f"ut{h}")
                     for h in range(2)]
            sums_ps = [ps_sum.tile([1, 512], f32, tag=f"sums{h}", name=f"sums{h}")
                       for h in range(2)]

            # ---- main k-loop over the gathered x ----
            # x_g block b rows [b*XR : b*XR+256] = x.T[:, b*1024:(b+1)*1024]
            for b in range(NSB):
                xt_t = [xtpool.tile([128, KC * SB], f16, tag=f"xt{h}", name=f"xt{h}")
                        for h in range(2)]
                for h in range(2):
                    nc.sync.dma_start(
                        xt_t[h][:],
                        x_g[b * XR + h * 128:b * XR + (h + 1) * 128, :])
                # natural-layout chunks via X-bar DMA transpose, then
                # cast fp16 -> bf16 (expT's dtype) on DVE
                xf_t = xfpool.tile([128, SB, D], f16, tag="xf", name="xf")
                for j in range(SB):
                    nc.sync.dma_start(
                        xf_t[:, j, :],
                        x_g[b * XR:b * XR + D, j * KC:(j + 1) * KC],
                        transpose=True)
                xn_t = xnpool.tile([128, SB, D], bf16, tag="xn", name="xn")
                nc.vector.tensor_copy(xn_t[:], xf_t[:])

                for j in range(SB):
                    c = b * SB + j
                    first, last = (c == 0), (c == N // KC - 1)
                    exps = []
                    for qh in range(2):
                        sp = ps_s.tile([128, 512], f32, tag="scores", name="scores")
                        for kp in range(2):
                            nc.tensor.matmul(
                                sp[:],
                                xt_t[kp][:, j * KC:(j + 1) * KC],
                                m_t[kp][:, qh * 512:(qh + 1) * 512],
                                start=(kp == 0), stop=(kp == 1),
                            )
                        et = epool.tile([128, 512], bf16, tag="expt", name="expt")
                        nc.scalar.activation(et[:], sp[:], EXP, bias=bias_t[:])
                        exps.append(et)
                    for qh in range(2):
                        et = exps[qh]
                        nc.tensor.matmul(
                            sums_ps[qh][:], ones_col[:], et[:],
                            start=first, stop=last)
                        for dh in range(2):
                            nc.tensor.matmul(
                                ut_ps[dh][:, qh * 512:(qh + 1) * 512],
                                xn_t[:, j, dh * 128:(dh + 1) * 128],
                                et[:],
                                start=first, stop=last)

            # ---- tail: softmax normalize + WV projection ----
            sums_sb = tpool.tile([1, NL], f32, name="sums_sb")
            for qh in range(2):
                nc.vector.tensor_copy(
                    sums_sb[:, qh * 512:(qh + 1) * 512], sums_ps[qh][:])
            recip_sb = tpool.tile([1, NL], f32r, name="recip_sb")
            with nc.allow_low_precision(reason="f32r is 4-byte, same mantissa path"):
                nc.vector.reciprocal(recip_sb[:], sums_sb[:])

            rb_sb = tpool.tile([128, NL], f32, name="rb_sb")
            for qh in range(2):
                rp = ps_s.tile([128, 512], f32, tag="scores", name="scores")
                nc.tensor.matmul(
                    rp[:], ones_row[:].bitcast(f32r),
                    recip_sb[:, qh * 512:(qh + 1) * 512],
                    start=True, stop=True)
                nc.vector.tensor_copy(rb_sb[:, qh * 512:(qh + 1) * 512], rp[:])

            utn_sb = [tpool.tile([128, NL], f16, tag=f"utn{h}", name=f"utn{h}")
                      for h in range(2)]
            with nc.allow_low_precision(reason="fp16 feeds fp32-accum matmul"):
                for dh in range(2):
                    nc.vector.tensor_mul(utn_sb[dh][:], ut_ps[dh][:], rb_sb[:])

            o_sb = [tpool.tile([128, NL], f16, tag=f"osb{h}", name=f"osb{h}")
                    for h in range(2)]
            for mh in range(2):
                op = ps_ut.tile([128, NL], f32, tag=f"ut{mh}", name=f"ut{mh}")
                for nh in range(2):
                    for kp in range(2):
                        nc.tensor.matmul(
                            op[:, nh * 512:(nh + 1) * 512],
                            wv_t[kp][:, mh * 128:(mh + 1) * 128],
                            utn_sb[kp][:, nh * 512:(nh + 1) * 512],
                            start=(kp == 0), stop=(kp == 1),
                        )
                nc.vector.tensor_copy(o_sb[mh][:], op[:])
                nc.sync.dma_start(outT[mh * 128:(mh + 1) * 128, :], o_sb[mh][:])

    nc.compile()
    return nc


def _get_nc():
    if "nc" not in _CACHE:
        _CACHE["nc"] = _build()
    return _CACHE["nc"]


def _in_maps(input, WQ, WK, WV):
    x = np.asarray(input, dtype=np.float32)
    wpack = np.concatenate([
        np.asarray(WQ, np.float32) @ np.asarray(WK, np.float32).T,
        np.asarray(WV, np.float32),
    ]).astype(np.float16).reshape(P, WS * D // 1024, 1024)
    g = np.empty((P, XR, NL), np.float16)
    # [D, N] -> [D, P, NL] -> [P, D, NL], cast fused into the assignment
    g[:, :D, :] = x.T.reshape(D, P, NL).swapaxes(0, 1)
    g[:, D:, :] = wpack
    return [{"xin": g[c]} for c in range(P)]


def kernel(input, WQ, WK, WV):
    _config_jax_cache()
    import time
    from concourse import bass_utils

    nc = _get_nc()
    in_maps = _in_maps(input, WQ, WK, WV)
    try:
        res = bass_utils.run_bass_kernel_spmd(
            nc, in_maps, core_ids=list(range(P)))
    except Exception:
        # transient device hiccup (e.g. NRT_EXEC_UNIT_UNRECOVERABLE after a
        # prior crashed run) — one retry after a short backoff
        time.sleep(2.0)
        res = bass_utils.run_bass_kernel_spmd(
            nc, in_maps, core_ids=list(range(P)))
    out = np.empty((N, D), dtype=np.float32)
    for c in range(P):
        out[c * NL:(c + 1) * NL, :] = res.results[c]["outT"].T
    return out


# revision 6
# speedup vs baseline: 1.2002x; 1.2002x over previous
"""Sequence-parallel self-attention for 8 TRN2 cores — minimal wire bytes.

Reference (N=8192, D=256, fp32):
    q = x @ WQ; k = x @ WK; v = x @ WV
    out = softmax(q @ k.T) @ v

The graded metric is wall-clock of kernel(), dominated by host->device
transfer over the axon tunnel (which compresses, so random fp32 is ~4x
the wire cost of fp16) plus per-call jit/compile overhead. Design:

  * ONE packed fp16 input per core (0.53 MiB vs 17.8 MiB baseline):
      xin [272, 1024] = [ x.T columns for my 1024 rows  (256 rows)
                        ; my 64-row shard of W = concat(WQ@WK.T, WV),
                          reshaped [64,256]->[16,1024]                ]
    Full x is assembled on-device with a single 8-core AllGather.
  * Output outT [257, 1024] int8 (out.T for my rows, quantized with
    per-d-row adaptive scales; the 256 f32 scales ride in row 256).
    Adds <= max|out|/254 ~ 3.9e-3 error, halves the return wire, and
    measured faster than fp16 out in a paired A/B.
  * JAX persistent compilation cache: run_bass_kernel_spmd re-jits a
    fresh closure every call, which otherwise re-runs the whole
    BIR->NEFF backend (~0.25s/call).

Per-core algebra (scores stay transposed so the softmax k-reduction is a
partition-axis ones-matmul):
    M  = (WQ @ WK.T).T @ xT_local                    [256, 1024]
    per k-chunk c (64 chunks of 128):
      scoresT = x_c @ M                              [128, 1024]
      expT    = exp(scoresT - 15)   (bf16)
      sums   += ones.T @ expT                        [1, 1024]
      UT     += x_c.T @ expT                         [256, 1024]
    outT = WV.T @ (UT / sums)                        [256, 1024]

x_c.T (natural-layout chunks, the UT lhsT) is derived on-chip from the
gathered x.T via X-bar DMA transpose (fp16, DRAM->SBUF) + a DVE cast to
bf16 (to match expT; expT can't be fp16 — exp(s-15) reaches ~1.2e11).

Numerics: fp16 score path end-to-end measures 3.3e-3 rel err in a
bit-accurate numpy model (bf16 would be 2.3e-2, over the 2e-2 gate).
All PSUM accumulation fp32.
"""

import numpy as np

N, D, P = 8192, 256, 8
NL = N // P          # 1024 q-rows per core
KC = 128             # k-chunk size
SB = 8               # k-chunks per superblock (= one AllGather block)
NSB = N // (KC * SB)  # 8 superblocks
WS = 2 * D // P      # 64 W-rows per core
XR = D + WS * D // 1024  # 272 packed input rows
EXP_SHIFT = -15.0

_CACHE = {}


def _config_jax_cache():
    if _CACHE.get("jax_cfg"):
        return
    _CACHE["jax_cfg"] = True
    try:
        import jax
        if jax.config.jax_compilation_cache_dir is None:
            jax.config.update("jax_compilation_cache_dir",
                              "/tmp/.bass_jax_comp_cache")
            jax.config.update("jax_persistent_cache_min_compile_time_secs", 0.0)
            jax.config.update("jax_persistent_cache_min_entry_size_bytes", 0)
    except Exception:
        pass


def _build():
    import concourse.bacc as bacc
    import concourse.mybir as mybir
    import concourse.tile as tile

    f32 = mybir.dt.float32
    f32r = mybir.dt.float32r
    f16 = mybir.dt.float16
    bf16 = mybir.dt.bfloat16
    EXP = mybir.ActivationFunctionType.Exp

    nc = bacc.Bacc("TRN2", target_bir_lowering=False, debug=False,
                   enable_asserts=False, num_devices=P)

    i8 = mybir.dt.int8
    xin = nc.dram_tensor("xin", [XR, NL], f16, kind="ExternalInput").ap()
    # rows 0:256 = int8-quantized out.T; row 256 = the 256 per-row f32
    # scales (abs-max per d-row) bitcast to 1024 int8 bytes
    outT = nc.dram_tensor("outT", [D + 1, NL], i8, kind="ExternalOutput").ap()

    rg = [list(range(P))]
    BYP = mybir.AluOpType.bypass

    with tile.TileContext(nc) as tc:
        with (
            tc.tile_pool(name="dram", bufs=1, space="DRAM") as dpool,
            tc.tile_pool(name="const", bufs=1) as cpool,
            tc.tile_pool(name="proj", bufs=1) as ppool,
            tc.tile_pool(name="xts", bufs=4) as xtpool,
            tc.tile_pool(name="xfs", bufs=2) as xfpool,
            tc.tile_pool(name="xns", bufs=4) as xnpool,
            tc.tile_pool(name="expt", bufs=8) as epool,
            tc.tile_pool(name="tail", bufs=1) as tpool,
            tc.tile_pool(name="ps_scores", bufs=2, space="PSUM") as ps_s,
            tc.tile_pool(name="ps_ut", bufs=1, space="PSUM") as ps_ut,
            tc.tile_pool(name="ps_sums", bufs=1, space="PSUM") as ps_sum,
        ):
            # ---- AllGather the packed shard into full-x DRAM ----
            x_b = dpool.tile([XR, NL], f16, name="x_b")
            x_g = dpool.tile([P * XR, NL], f16, addr_space="Shared", name="x_g")
            nc.sync.dma_start(x_b[:], xin[:])
            nc.gpsimd.collective_compute(
                "AllGather", BYP, replica_groups=rg,
                ins=[x_b.opt()], outs=[x_g.opt()])

            # ---- weights from the gathered blocks ----
            # W row r (256 wide) lives in shard c=r//64's block, rows
            # [c*XR+256 : c*XR+272] as [16, 1024] (4 W-rows per row).
            wkqt_t = [cpool.tile([128, D], f16, tag=f"wkqt{h}", name=f"wkqt{h}")
                      for h in range(2)]
            wv_t = [cpool.tile([128, D], f16, tag=f"wv{h}", name=f"wv{h}")
                    for h in range(2)]
            for t, base in ((wkqt_t, 0), (wv_t, D)):
                for kp in range(2):
                    for s in range(2):
                        c = (base + kp * 128) // WS + s
                        nc.sync.dma_start(
                            t[kp][s * 64:(s + 1) * 64, :],
                            x_g[c * XR + D:(c + 1) * XR, :]
                            .rearrange("p (a d) -> (p a) d", a=4))

            xtl_t = [cpool.tile([128, NL], f16, tag=f"xtl{h}", name=f"xtl{h}")
                     for h in range(2)]
            for h in range(2):
                nc.sync.dma_start(xtl_t[h][:], xin[h * 128:(h + 1) * 128, :])
            ones_col = cpool.tile([128, 1], bf16, name="ones_col")
            ones_row = cpool.tile([1, 128], f32, name="ones_row")
            bias_t = cpool.tile([128, 1], f32, name="bias_t")
            nc.gpsimd.memset(ones_col[:], 1.0)
            nc.vector.memset(ones_row[:], 1.0)
            nc.vector.memset(bias_t[:], EXP_SHIFT)

            # ---- M = WKQ @ xT_local  (lhsT = WKQT = WQ @ WK.T) ----
            m_t = [ppool.tile([128, NL], f16, tag=f"m{h}", name=f"m{h}")
                   for h in range(2)]
            for mh in range(2):
                for nh in range(2):
                    pp = ps_s.tile([128, 512], f32, tag="scores", name="scores")
                    for kp in range(2):
                        nc.tensor.matmul(
                            pp[:],
                            wkqt_t[kp][:, mh * 128:(mh + 1) * 128],
                            xtl_t[kp][:, nh * 512:(nh + 1) * 512],
                            start=(kp == 0), stop=(kp == 1),
                        )
                    nc.vector.tensor_copy(
                        m_t[mh][:, nh * 512:(nh + 1) * 512], pp[:])

            # ---- persistent accumulators ----
            ut_ps = [ps_ut.tile([128, NL], f32, tag=f"ut{h}", name=f"ut{h}")
                     for h in range(2)]
            sums_ps = [ps_sum.tile([1, 512], f32, tag=f"sums{h}", name=f"sums{h}")
                       for h in range(2)]

            # ---- main k-loop over the gathered x ----
            # x_g block b rows [b*XR : b*XR+256] = x.T[:, b*1024:(b+1)*1024]
            for b in range(NSB):
                xt_t = [xtpool.tile([128, KC * SB], f16, tag=f"xt{h}", name=f"xt{h}")
                        for h in range(2)]
                for h in range(2):
                    nc.sync.dma_start(
                        xt_t[h][:],
                        x_g[b * XR + h * 128:b * XR + (h + 1) * 128, :])
                # natural-layout chunks via X-bar DMA transpose, then
                # cast fp16 -> bf16 (expT's dtype) on DVE
                xf_t = xfpool.tile([128, SB, D], f16, tag="xf", name="xf")
                for j in range(SB):
                    nc.sync.dma_start(
                        xf_t[:, j, :],
                        x_g[b * XR:b * XR + D, j * KC:(j + 1) * KC],
                        transpose=True)
                xn_t = xnpool.tile([128, SB, D], bf16, tag="xn", name="xn")
                nc.vector.tensor_copy(xn_t[:], xf_t[:])

                for j in range(SB):
                    c = b * SB + j
                    first, last = (c == 0), (c == N // KC - 1)
                    exps = []
                    for qh in range(2):
                        sp = ps_s.tile([128, 512], f32, tag="scores", name="scores")
                        for kp in range(2):
                            nc.tensor.matmul(
                                sp[:],
                                xt_t[kp][:, j * KC:(j + 1) * KC],
                                m_t[kp][:, qh * 512:(qh + 1) * 512],
                                start=(kp == 0), stop=(kp == 1),
                            )
                        et = epool.tile([128, 512], bf16, tag="expt", name="expt")
                        nc.scalar.activation(et[:], sp[:], EXP, bias=bias_t[:])
                        exps.append(et)
                    for qh in range(2):
                        et = exps[qh]
                        nc.tensor.matmul(
                            sums_ps[qh][:], ones_col[:], et[:],
                            start=first, stop=last)
                        for dh in range(2):
                            nc.tensor.matmul(
                                ut_ps[dh][:, qh * 512:(qh + 1) * 512],
                                xn_t[:, j, dh * 128:(dh + 1) * 128],
                                et[:],
                                start=first, stop=last)

            # ---- tail: softmax normalize + WV projection ----
            sums_sb = tpool.tile([1, NL], f32, name="sums_sb")
            for qh in range(2):
                nc.vector.tensor_copy(
                    sums_sb[:, qh * 512:(qh + 1) * 512], sums_ps[qh][:])
            recip_sb = tpool.tile([1, NL], f32r, name="recip_sb")
            with nc.allow_low_precision(reason="f32r is 4-byte, same mantissa path"):
                nc.vector.reciprocal(recip_sb[:], sums_sb[:])

            rb_sb = tpool.tile([128, NL], f32, name="rb_sb")
            for qh in range(2):
                rp = ps_s.tile([128, 512], f32, tag="scores", name="scores")
                nc.tensor.matmul(
                    rp[:], ones_row[:].bitcast(f32r),
                    recip_sb[:, qh * 512:(qh + 1) * 512],
                    start=True, stop=True)
                nc.vector.tensor_copy(rb_sb[:, qh * 512:(qh + 1) * 512], rp[:])

            utn_sb = [tpool.tile([128, NL], f16, tag=f"utn{h}", name=f"utn{h}")
                      for h in range(2)]
            with nc.allow_low_precision(reason="fp16 feeds fp32-accum matmul"):
                for dh in range(2):
                    nc.vector.tensor_mul(utn_sb[dh][:], ut_ps[dh][:], rb_sb[:])

            # int8 output with per-row (per-d) adaptive scales packed into the
            # same tensor: wire halves; added error <= max|out|/254 ~ 3.9e-3
            # regardless of data.
            for mh in range(2):
                op = ps_ut.tile([128, NL], f32, tag=f"ut{mh}", name=f"ut{mh}")
                for nh in range(2):
                    for kp in range(2):
                        nc.tensor.matmul(
                            op[:, nh * 512:(nh + 1) * 512],
                            wv_t[kp][:, mh * 128:(mh + 1) * 128],
                            utn_sb[kp][:, nh * 512:(nh + 1) * 512],
                            start=(kp == 0), stop=(kp == 1),
                        )
                rmax = tpool.tile([128, 1], f32, tag=f"rmax{mh}", name=f"rmax{mh}")
                nc.vector.tensor_reduce(
                    rmax[:], op[:], axis=mybir.AxisListType.X,
                    op=mybir.AluOpType.max, apply_absolute_value=True)
                nc.vector.tensor_scalar_max(rmax[:], rmax[:], 1e-30)
                rinv = tpool.tile([128, 1], f32, tag=f"rinv{mh}", name=f"rinv{mh}")
                nc.vector.reciprocal(rinv[:], rmax[:])
                nc.vector.tensor_scalar_mul(rinv[:], rinv[:], 127.0)
                o8 = tpool.tile([128, NL], i8, tag=f"o8{mh}", name=f"o8{mh}")
                with nc.allow_low_precision(reason="int8 wire quantization"):
                    nc.vector.tensor_scalar_mul(o8[:], op[:], rinv[:])
                nc.sync.dma_start(outT[mh * 128:(mh + 1) * 128, :], o8[:])
                nc.sync.dma_start(
                    outT[D:D + 1, mh * 512:(mh + 1) * 512].bitcast(f32),
                    rmax[:])

    nc.compile()
    return nc


def _get_nc():
    if "nc" not in _CACHE:
        _CACHE["nc"] = _build()
    return _CACHE["nc"]


def _in_maps(input, WQ, WK, WV):
    x = np.asarray(input, dtype=np.float32)
    wpack = np.concatenate([
        np.asarray(WQ, np.float32) @ np.asarray(WK, np.float32).T,
        np.asarray(WV, np.float32),
    ]).astype(np.float16).reshape(P, WS * D // 1024, 1024)
    g = np.empty((P, XR, NL), np.float16)
    # [D, N] -> [D, P, NL] -> [P, D, NL], cast fused into the assignment
    g[:, :D, :] = x.T.reshape(D, P, NL).swapaxes(0, 1)
    g[:, D:, :] = wpack
    return [{"xin": g[c]} for c in range(P)]


def kernel(input, WQ, WK, WV):
    _config_jax_cache()
    import time
    from concourse import bass_utils

    nc = _get_nc()
    in_maps = _in_maps(input, WQ, WK, WV)
    try:
        res = bass_utils.run_bass_kernel_spmd(
            nc, in_maps, core_ids=list(range(P)))
    except Exception:
        # transient device hiccup (e.g. NRT_EXEC_UNIT_UNRECOVERABLE after a
        # prior crashed run) — one retry after a short backoff
        time.sleep(2.0)
        res = bass_utils.run_bass_kernel_spmd(
            nc, in_maps, core_ids=list(range(P)))
    out = np.empty((N, D), dtype=np.float32)
    for c in range(P):
        raw = res.results[c]["outT"]
        o8 = raw[:D].astype(np.float32)
        sc = raw[D].copy().view(np.float32) * (1.0 / 127.0)
        out[c * NL:(c + 1) * NL, :] = (o8 * sc[:, None]).T
    return out
